# revision 11
# baseline (speedup 1.0000x reference)
"""Trainium2 Bass kernel for nn_Dconv_drop (randomized-sample 3x3 conv).

Math: out[b,o,h,w] = sum_{c,k} weight[o,c,k] * x[b,c,flat_idx(h,w,k)]
  x: [32,64,56,56] f32, weight: [64,64,3,3] f32, sample_idx: [56,56,9] i32.

Strategy (8 cores, data-parallel over batch, 4 images/core):
  Host packs x into pixel-major bf16 rows x4[s, (pair,img,c)] (512B rows, the
  full-rate DMA gather element size) and uploads it as an input, so the device
  does no pre-pass at all:
  1. dma_gather (GPSIMD SWDGE, transpose mode) straight from the x4 DRAM
     input: per hw-tile, gather the 9 taps' source rows; output lands as
     [128=(b_even c | b_odd c), pair, slots] bf16 -- the matmul moving operand.
  2. TensorE: per image-pair, 9 accumulating matmuls with block-diagonal
     weights produce out[(o,b_even | o,b_odd), hw] in PSUM (f32).
  3. ACT/DVE copy PSUM -> bf16 staging, DMA -> HBM out (bf16, host upcasts).
"""

import os
import threading

import numpy as np
import ml_dtypes

B, CIN, COUT = 32, 64, 64
H = W = 56
HW = H * W              # 3136
K9 = 9
NCORES = 8
BPC = B // NCORES       # 4 images per core
NPAIR = BPC // 2        # 2 image pairs -> 128-partition packing
TILE = 640              # hw pixels per gather tile (9*640 % 128 == 0)
NTILES = (HW + TILE - 1) // TILE  # 5
IDX_PER_TILE = K9 * TILE          # 5760
IDX_COLS = IDX_PER_TILE // 16     # 360
SUB = 320               # psum sub-tile columns (<=512 f32 / bank)
NSUB = TILE // SUB      # 2
HWPAD = 3200            # x4 rows padded to 25 ranks of 128
NRANK = HWPAD // 128    # 25
# SBUF x4 load chunks (in ranks of 128 rows): gather tile t touches source
# rows within +-399 of its own pixel range, so it only needs ranks < RANK_HI[t]
RANK_HI = [9, 14, 19, 24, 25]

_lock = threading.Lock()
_cache: dict = {}


def _build_program():
    import concourse.bass as bass  # noqa: F401
    import concourse.bacc as bacc
    import concourse.mybir as mybir
    import concourse.tile as tile

    fp32 = mybir.dt.float32
    bf16 = mybir.dt.bfloat16
    i16 = mybir.dt.int16

    # Bacc (not raw Bass): its compile() legalizes semaphore waits --
    # hardware allows at most one sync wait per engine instruction
    nc = bacc.Bacc()
    x4_in = nc.dram_tensor("x4", (HWPAD, BPC * CIN), bf16, kind="ExternalInput")
    w_in = nc.dram_tensor("wT", (128, K9 * 128), bf16, kind="ExternalInput")
    idx_in = nc.dram_tensor("idx", (128, NTILES * IDX_COLS), i16,
                            kind="ExternalInput")
    out = nc.dram_tensor("out", (BPC * COUT, HW), bf16, kind="ExternalOutput")

    with tile.TileContext(nc) as tc:
        with (
            tc.tile_pool(name="const", bufs=1) as constp,
            tc.tile_pool(name="stage", bufs=2) as stage,
            tc.tile_pool(name="gath", bufs=3) as gath,
        ):
            idx_sb = constp.tile([128, NTILES * IDX_COLS], i16)
            # tile-0 indices first so gather 0's desc-gen starts early
            idx_dma0 = nc.sync.dma_start(idx_sb[:, 0:IDX_COLS],
                                         idx_in[:, 0:IDX_COLS])
            # x4 pixel rows in SBUF, stripe layout for the SBUF-source gather:
            # x4_sb[p, r*256:(r+1)*256] = x4[r*128 + p, :]  (512B rank stripes)
            x4_sb = constp.tile([128, NRANK, BPC * CIN], bf16)
            x4_v = x4_in[:].rearrange("(r p) e -> p r e", p=128)
            x4_dmas = []
            qs = [nc.scalar, nc.sync]
            r0 = 0
            for ci, r1 in enumerate(RANK_HI):
                x4_dmas.append(qs[ci % 2].dma_start(
                    x4_sb[:, r0:r1, :], x4_v[:, r0:r1, :]))
                r0 = r1
            idx_dmaR = nc.sync.dma_start(idx_sb[:, IDX_COLS:],
                                         idx_in[:, IDX_COLS:])
            w_sb = constp.tile([128, K9 * 128], bf16)
            nc.scalar.dma_start(w_sb[:], w_in[:])
            # scratch target for the wait-absorber memsets: each memset eats
            # one producer's semaphore wait on the Pool engine so the
            # wait-slot-limited DMAGather instructions carry none themselves.
            # Every absorber writes its own column -- any WAW overlap would
            # add a Pool self-wait and bust the 1-wait-per-instruction limit.
            scratch = constp.tile([128, 16], bf16)
            scratch_col = [0]

            pmm_cm = tc.tile_pool(name="pmm", bufs=4, space="PSUM")
            pmm = pmm_cm.__enter__()
            gathers = []
            tile_last_mm = []
            GBUFS = 3  # gath pool bufs
            for t in range(NTILES):
                # every gather t needs its x4 band chunk; chunks < t were
                # already absorbed by earlier gathers (Pool is in-order)
                absorb_deps = [x4_dmas[t]]
                if t == 0:
                    absorb_deps.append(idx_dma0)
                elif t == 1:
                    absorb_deps.append(idx_dmaR)
                if t >= GBUFS:
                    # g slot recycle: previous writer (gather) + last reader
                    # (final matmul) of the tile GBUFS back
                    absorb_deps += [gathers[t - GBUFS], tile_last_mm[t - GBUFS]]
                last_abs = None
                for d in absorb_deps:
                    if d is None:
                        continue
                    col = scratch_col[0]
                    scratch_col[0] += 1
                    m = nc.gpsimd.memset(scratch[:, col:col + 1], 0)
                    tile.add_dep_helper(m.ins, d.ins, sync=True,
                                        reason="gather wait absorber")
                    last_abs = m
                g = gath.tile([128, NPAIR, IDX_PER_TILE], bf16, tag="g")
                gather = nc.gpsimd.dma_gather(
                    out_ap=g[:],
                    in_ap=x4_sb[:],
                    idxs_ap=idx_sb[:, t * IDX_COLS:(t + 1) * IDX_COLS],
                    num_idxs=IDX_PER_TILE,
                    num_idxs_reg=IDX_PER_TILE,
                    elem_size=BPC * CIN,
                    transpose=True,
                    # single_packet=True silently caps a transpose gather
                    # around ~512 indices on hardware (probed: 512 OK, 1024
                    # faults); multi-packet handles our 5760-index tiles
                    single_packet=False,
                    sbuf_tokens_per_rank=128,
                    sbuf_free_dim_per_rank=BPC * CIN * 2,
                )
                if last_abs is not None:
                    tile.add_dep_helper(gather.ins, last_abs.ins, sync=False,
                                        reason="absorbers before gather")
                gathers.append(gather)
                # dummy weight load: a PE instruction whose only dependency
                # is the gather -- it absorbs the SWDGE-sem wait so the real
                # matmuls (which also wait on their PSUM slot release) stay
                # within the 1-sync-wait-per-PE-instruction ISA limit.
                sentinel = nc.tensor.ldweights(g[:, 0, 0:128])
                first_mm_of_tile = None
                last_mm = None
                for j in range(NPAIR):
                    tvalid = min(TILE, HW - t * TILE)
                    ob = stage.tile([128, TILE], bf16, tag="ob")
                    for s in range(NSUB):
                        lo = t * TILE + s * SUB          # global hw start
                        valid = max(0, min(SUB, HW - lo))
                        if valid == 0:
                            continue
                        acc = pmm.tile([128, SUB], fp32, tag="acc")
                        for k in range(K9):
                            mm = nc.tensor.matmul(
                                acc[:],
                                w_sb[:, k * 128:(k + 1) * 128],
                                g[:, j, k * TILE + s * SUB:k * TILE + s * SUB + SUB],
                                start=(k == 0),
                                stop=(k == K9 - 1),
                            )
                            last_mm = mm
                            if k == 0 and first_mm_of_tile is None:
                                first_mm_of_tile = mm
                                tile.add_dep_helper(
                                    mm.ins, sentinel.ins, sync=False,
                                    reason="order gather-sentinel before mm",
                                )
                        if (j + s) % 2 == 0:
                            nc.scalar.copy(ob[:, s * SUB:s * SUB + valid],
                                           acc[:, 0:valid])
                        else:
                            nc.vector.tensor_copy(
                                ob[:, s * SUB:s * SUB + valid],
                                acc[:, 0:valid])
                    # rows (2j*64 + p) of out_flat are contiguous: one
                    # 128-partition DMA covers both images of the pair
                    eng = (nc.sync, nc.scalar)[(t + j) % 2]
                    eng.dma_start(
                        out[2 * j * COUT:2 * j * COUT + 128,
                            t * TILE:t * TILE + tvalid],
                        ob[:, 0:tvalid])
                tile_last_mm.append(last_mm)
            pmm_cm.__exit__(None, None, None)
    nc.compile()
    return nc


def _host_prep(weight: np.ndarray, sample_idx: np.ndarray):
    """Build the weight lhsT and wrapped gather indices."""
    w9 = weight.reshape(COUT, CIN, K9).astype(ml_dtypes.bfloat16)
    wT = np.zeros((128, K9 * 128), dtype=ml_dtypes.bfloat16)
    for k in range(K9):
        # lhsT[K=(c|c), M=(o_even|o_odd)] block-diagonal
        wT[0:CIN, k * 128:k * 128 + COUT] = w9[:, :, k].T
        wT[CIN:128, k * 128 + COUT:(k + 1) * 128] = w9[:, :, k].T

    si = sample_idx.reshape(HW, K9).astype(np.int64)  # [hw, k]
    idx_all = np.zeros((128, NTILES * IDX_COLS), dtype=np.int16)
    for t in range(NTILES):
        slots = np.zeros(IDX_PER_TILE, dtype=np.int16)
        for k in range(K9):
            lo = t * TILE
            hi = min(lo + TILE, HW)
            if hi > lo:
                band = si[lo:hi, k]
                assert band.min() >= 0 and band.max() < RANK_HI[t] * 128
                slots[k * TILE:k * TILE + (hi - lo)] = band
        wrapped = np.zeros((16, IDX_COLS), dtype=np.int16)
        ii = np.arange(IDX_PER_TILE)
        wrapped[ii % 16, ii // 16] = slots
        idx_all[:, t * IDX_COLS:(t + 1) * IDX_COLS] = np.tile(wrapped, (8, 1))

    return wT, idx_all


def kernel(x: np.ndarray, weight: np.ndarray, sample_idx: np.ndarray
           ) -> np.ndarray:
    from concourse.bass_utils import run_bass_kernel_spmd

    x = np.ascontiguousarray(np.asarray(x, dtype=np.float32))
    weight = np.asarray(weight, dtype=np.float32)
    sample_idx = np.asarray(sample_idx, dtype=np.int32)

    with _lock:
        if "nc" not in _cache:
            _cache["nc"] = _build_program()
        nc = _cache["nc"]

    wT, idx_all = _host_prep(weight, sample_idx)
    # pixel-major pack: x4[core][s, (pair, img_in_pair, c)] bf16
    xb = x.reshape(B, CIN, HW).astype(ml_dtypes.bfloat16)
    in_maps = []
    for c in range(NCORES):
        shard = xb[c * BPC:(c + 1) * BPC]            # [4, 64, HW]
        x4 = np.zeros((HWPAD, BPC * CIN), dtype=ml_dtypes.bfloat16)
        x4[:HW] = shard.transpose(2, 0, 1).reshape(HW, BPC * CIN)
        in_maps.append({
            "x4": x4,
            "wT": wT,
            "idx": idx_all,
        })

    trace = bool(int(os.environ.get("KERNEL_TRACE", "0")))
    res = run_bass_kernel_spmd(nc, in_maps, core_ids=list(range(NCORES)),
                               trace=trace)
    if trace:
        _cache["last_result"] = res

    out = np.empty((B, COUT, HW), dtype=np.float32)
    for c in range(NCORES):
        out[c * BPC:(c + 1) * BPC] = res.results[c]["out"].astype(
            np.float32).reshape(BPC, COUT, HW)
    return out.reshape(B, COUT, H, W)
